# revision 11
# baseline (speedup 1.0000x reference)
"""DyDCNv2 (modulated deformable conv 3x3 + GroupNorm) on 8 Trainium2 cores.

Sharding: core c handles batch b=c//2, row-half h=c%2 (48 of 96 rows).
Per core the full per-batch input image is kept in DRAM as a row-major
[H*W, C] table (front pad 1, back pad 3 rows; indices shifted +1).

v3: tuned for the measured per-instruction cost structure of this stack
(dominant fixed dispatch cost per instruction; SWDGE gather cost is strongly
superlinear in descriptor count above ~256; matmul cost ~ FLOPs at sgemm
rate with fp32 noticeably faster than bf16).  Consequences:
- gathers are 256-descriptor non-transpose SWDGE ops, each descriptor
  fetching the x0/x0+1 channel-vector pair (1KB contiguous);
- corner weights AND gather indices are computed in one fused setup pass
  ([128,NT,NS] pixel-partition layout + a tiny [16,NT,288] wrap layout for
  the SWDGE index tables; no PE transposes, no DRAM bounces);
- the 4-corner bilinear blend runs as 3 big DVE ops per quarter-row-block
  (one fused multiply over all 4 corners via strided/broadcast APs);
- the [pix,c] -> [c,pix] transpose is a DRAM round-trip (contiguous write,
  strided read) instead of 648 PE transpose instructions;
- the conv matmuls and GroupNorm run in fp32 (faster sgemm path).
"""

import os
import numpy as np
import ml_dtypes

import concourse.bass as bass
import concourse.bacc as bacc
import concourse.mybir as mybir
import concourse.tile as tile
from concourse import library_config
from concourse.bass_utils import run_bass_kernel_spmd

P = 128
B, CIN, COUT, H, W = 4, 256, 256, 96, 96
HP = H // 2              # output rows per core
NPIX = HP * W            # 4608 output pixels per core
NT = 9                   # 3x3 taps
R = H * W + 4            # gather table rows (1 front + 3 back pad; idx shifted +1)
NS = NPIX // P           # 36 slots: pixel = s*128 + p
NC8 = 18                 # 256-px gather chunks per tap
MQ = NPIX // 16          # 288 idx-wrap columns (16 partitions)
GN_GROUPS = 16
EPS = 1e-5
NG = (CIN // GN_GROUPS) * (H * W)  # elements per GN group (full image)

F32 = mybir.dt.float32
BF16 = mybir.dt.bfloat16
I16 = mybir.dt.int16
AX = mybir.AxisListType
OP = mybir.AluOpType

NOGATHER = os.environ.get("DCN_NOGATHER") == "1"
ITERS = int(os.environ.get("DCN_ITERS", "1"))
NOCC = os.environ.get("DCN_NOCC") == "1"
DTM = BF16               # gather table dtype
NPDT = ml_dtypes.bfloat16

_CACHED = {}


def _build_nc():
    nc = bacc.Bacc("TRN2", target_bir_lowering=False, debug=False, num_devices=8)

    xt = nc.dram_tensor("xt", [R * CIN], DTM, kind="ExternalInput")
    # pixel-partition layout inputs: [P, NT, NS], pixel = s*128 + p
    offs_py = nc.dram_tensor("offs_py", [P, NT, NS], F32, kind="ExternalInput")
    offs_px = nc.dram_tensor("offs_px", [P, NT, NS], F32, kind="ExternalInput")
    msk_pw = nc.dram_tensor("msk_pw", [P, NT, NS], F32, kind="ExternalInput")
    bw_y = nc.dram_tensor("bw_y", [P, NT, NS], F32, kind="ExternalInput")
    bw_x = nc.dram_tensor("bw_x", [P, NT, NS], F32, kind="ExternalInput")
    # idx-wrap layout inputs: [16, NT, MQ], pixel = (M//32)*512 + (M%32)*16 + q
    offs_iy = nc.dram_tensor("offs_iy", [16, NT, MQ], F32, kind="ExternalInput")
    offs_ix = nc.dram_tensor("offs_ix", [16, NT, MQ], F32, kind="ExternalInput")
    bi_y = nc.dram_tensor("bi_y", [16, NT, MQ], F32, kind="ExternalInput")
    bi_x = nc.dram_tensor("bi_x", [16, NT, MQ], F32, kind="ExternalInput")
    wtd = nc.dram_tensor("wtd", [NT, CIN, COUT], F32, kind="ExternalInput")
    ind8d = nc.dram_tensor("ind8d", [P, 8], F32, kind="ExternalInput")
    e8d = nc.dram_tensor("e8d", [8, P], F32, kind="ExternalInput")
    gamd = nc.dram_tensor("gamd", [P, 2], F32, kind="ExternalInput")
    betd = nc.dram_tensor("betd", [P, 2], F32, kind="ExternalInput")
    yout = nc.dram_tensor("y", [COUT, NPIX], F32, kind="ExternalOutput")
    DEBUG = os.environ.get("DCN_DEBUG") == "1"
    if DEBUG:
        dbg_idx = [nc.dram_tensor(f"dbg_idx{s_}", [P, NT, MQ], I16,
                                  kind="ExternalOutput") for s_ in range(2)]
        dbg_cw = nc.dram_tensor("dbg_cw", [P, 2, 2, NT, NS], F32, kind="ExternalOutput")
        dbg_y = nc.dram_tensor("dbg_y", [P, 2, NPIX], F32, kind="ExternalOutput")
        dbg_g = [nc.dram_tensor(f"dbg_g{k}", [P, 18, 2 * CIN], DTM,
                                kind="ExternalOutput") for k in range(2)]
        dbg_m = nc.dram_tensor("dbg_m", [P, 18, CIN], F32, kind="ExternalOutput")
        dbg_mt = nc.dram_tensor("dbg_mt", [P, 2, 1536], F32, kind="ExternalOutput")

    nc.gpsimd.load_library(library_config.mlp)

    # paired-corner gather source: each idx reads rows [idx, idx+1] (2*CIN elems)
    gather_src = bass.AP(xt[:].tensor, 0, [[CIN, R - 1], [1, 2 * CIN]])

    with tile.TileContext(nc) as tc:
        with (
            tc.tile_pool(name="const", bufs=1) as cp,
            tc.tile_pool(name="dram", bufs=1, space="DRAM") as dp,
        ):
            # ---------------- constants ----------------
            wtile = cp.tile([P, NT, 2, COUT], F32, tag="wtile")
            nc.sync.dma_start(wtile[:], wtd.rearrange("t (kc p) o -> p t kc o", p=P))
            ind8 = cp.tile([P, 8], F32, tag="ind8")
            nc.sync.dma_start(ind8[:], ind8d[:])
            e8 = cp.tile([8, P], F32, tag="e8")
            nc.sync.dma_start(e8[:], e8d[:])
            gam = cp.tile([P, 2], F32, tag="gam")
            nc.sync.dma_start(gam[:], gamd[:])
            bet = cp.tile([P, 2], F32, tag="bet")
            nc.sync.dma_start(bet[:], betd[:])

            # persistent per-iteration products
            # cwb[p, side, xc, t, s]: corner weight (y-side, x-corner) incl. mask
            cwb = cp.tile([P, 2, 2, NT, NS], F32, tag="cwb")
            # SWDGE idx tables (sim reads first 16 partitions only)
            idxs = [cp.tile([P, NT, MQ], I16, tag=f"idx{k}", name=f"idx{k}")
                    for k in range(2)]
            ysb = cp.tile([P, 2, NPIX], F32, tag="ysb")
            # DRAM bounce for the [pix,c] -> [c,pix] transpose
            md = dp.tile([NT * NPIX * CIN], F32, tag="md")

            for _it in range(ITERS):
                # ---------------- setup: corner weights + indices ----------------
                with tc.tile_pool(name="setup", bufs=1) as wk:
                    # --- corner weights, [P, NT, NS] layout ---
                    def build_w(off_d, base_d, lim, tg):
                        pyt = wk.tile([P, NT, NS], F32, tag=f"py{tg}", name=f"py{tg}")
                        nc.sync.dma_start(pyt[:], off_d[:])
                        bt = wk.tile([P, NT, NS], F32, tag=f"b{tg}", name=f"b{tg}")
                        nc.sync.dma_start(bt[:], base_d[:])
                        nc.vector.tensor_tensor(pyt[:], pyt[:], bt[:], op=OP.add)
                        fy = wk.tile([P, NT, NS], F32, tag=f"f{tg}", name=f"f{tg}")
                        gt = wk.tile([P, NT, NS], F32, tag=f"g{tg}", name=f"g{tg}")
                        # floor via round-to-nearest magic + compare correction
                        nc.vector.tensor_scalar(fy[:], pyt[:], 12582912.0, -12582912.0,
                                                op0=OP.add, op1=OP.add)
                        nc.vector.tensor_tensor(gt[:], fy[:], pyt[:], op=OP.is_gt)
                        y0 = bt  # reuse
                        nc.vector.tensor_tensor(y0[:], fy[:], gt[:], op=OP.subtract)
                        nc.vector.tensor_tensor(fy[:], pyt[:], y0[:], op=OP.subtract)
                        w0 = pyt  # reuse: w0 = 1 - frac
                        nc.vector.tensor_scalar(w0[:], fy[:], -1.0, 1.0, op0=OP.mult, op1=OP.add)
                        v0 = wk.tile([P, NT, NS], F32, tag=f"v0{tg}", name=f"v0{tg}")
                        vt = wk.tile([P, NT, NS], F32, tag=f"vt{tg}", name=f"vt{tg}")
                        nc.vector.tensor_scalar(v0[:], y0[:], 0.0, None, op0=OP.is_ge)
                        nc.vector.tensor_scalar(vt[:], y0[:], float(lim - 1), None, op0=OP.is_le)
                        nc.vector.tensor_tensor(v0[:], v0[:], vt[:], op=OP.mult)
                        v1 = wk.tile([P, NT, NS], F32, tag=f"v1{tg}", name=f"v1{tg}")
                        nc.vector.tensor_scalar(v1[:], y0[:], -1.0, None, op0=OP.is_ge)
                        nc.vector.tensor_scalar(vt[:], y0[:], float(lim - 2), None, op0=OP.is_le)
                        nc.vector.tensor_tensor(v1[:], v1[:], vt[:], op=OP.mult)
                        nc.vector.tensor_tensor(v0[:], w0[:], v0[:], op=OP.mult)
                        nc.vector.tensor_tensor(v1[:], fy[:], v1[:], op=OP.mult)
                        return v0, v1

                    cy0, cy1 = build_w(offs_py, bw_y, H, "y")
                    cx0, cx1 = build_w(offs_px, bw_x, W, "x")
                    mskt = wk.tile([P, NT, NS], F32, tag="mskt")
                    nc.sync.dma_start(mskt[:], msk_pw[:])
                    nc.vector.tensor_tensor(cy0[:], cy0[:], mskt[:], op=OP.mult)
                    nc.vector.tensor_tensor(cy1[:], cy1[:], mskt[:], op=OP.mult)
                    for side, cy in enumerate((cy0, cy1)):
                        for xc, cx in enumerate((cx0, cx1)):
                            nc.vector.tensor_tensor(cwb[:, side, xc], cy[:], cx[:],
                                                    op=OP.mult)

                    # --- indices, [16, NT, MQ] wrap layout ---
                    def build_floor16(off_d, base_d, tg):
                        pt = wk.tile([16, NT, MQ], F32, tag=f"p6{tg}", name=f"p6{tg}")
                        nc.sync.dma_start(pt[:], off_d[:])
                        bt = wk.tile([16, NT, MQ], F32, tag=f"b6{tg}", name=f"b6{tg}")
                        nc.sync.dma_start(bt[:], base_d[:])
                        nc.vector.tensor_tensor(pt[:], pt[:], bt[:], op=OP.add)
                        ft = wk.tile([16, NT, MQ], F32, tag=f"f6{tg}", name=f"f6{tg}")
                        gt = wk.tile([16, NT, MQ], F32, tag=f"g6{tg}", name=f"g6{tg}")
                        nc.vector.tensor_scalar(ft[:], pt[:], 12582912.0, -12582912.0,
                                                op0=OP.add, op1=OP.add)
                        nc.vector.tensor_tensor(gt[:], ft[:], pt[:], op=OP.is_gt)
                        nc.vector.tensor_tensor(pt[:], ft[:], gt[:], op=OP.subtract)
                        return pt

                    y06 = build_floor16(offs_iy, bi_y, "y")
                    x06 = build_floor16(offs_ix, bi_x, "x")
                    xb = x06
                    nc.vector.tensor_scalar(xb[:], x06[:], 0.0, float(W + 1),
                                            op0=OP.max, op1=OP.min)
                    for side in range(2):
                        yc = wk.tile([16, NT, MQ], F32, tag=f"yc{side}", name=f"yc{side}")
                        if side == 0:
                            nc.vector.tensor_scalar(yc[:], y06[:], 0.0, float(H - 1),
                                                    op0=OP.max, op1=OP.min)
                        else:
                            nc.vector.tensor_scalar(yc[:], y06[:], 1.0, None, op0=OP.add)
                            nc.vector.tensor_scalar(yc[:], yc[:], 0.0, float(H - 1),
                                                    op0=OP.max, op1=OP.min)
                        nc.vector.tensor_scalar(yc[:], yc[:], float(W), None, op0=OP.mult)
                        nc.vector.tensor_tensor(yc[:], yc[:], xb[:], op=OP.add)
                        nc.vector.tensor_scalar(yc[:], yc[:], 0.0, float(H * W + 2),
                                                op0=OP.max, op1=OP.min)
                        # the gather ucode reads per-Q7-core replicas from each
                        # 16-partition group: bounce through DRAM and replicate
                        nc.vector.tensor_copy(idxs[side][:16], yc[:])
                        ib = dp.tile([16, NT * MQ], I16, tag=f"ib{side}",
                                     name=f"ib{side}")
                        nc.sync.dma_start(ib[:], idxs[side][:16])
                        for g in range(1, 8):
                            nc.sync.dma_start(idxs[side][16 * g:16 * (g + 1)], ib[:])

                if DEBUG:
                    for s_ in range(2):
                        nc.sync.dma_start(dbg_idx[s_][:], idxs[s_][:])
                    nc.sync.dma_start(dbg_cw[:], cwb[:])

                # ---------------- gather + blend + transpose-bounce ----------------
                with tc.tile_pool(name="gat", bufs=1) as gp:
                    for t in range(NT):
                        for th in range(3):  # thirds of 1536 px (12 slots)
                            G = gp.tile([P, 2, 12, 2 * CIN], DTM, tag="G", name="G")
                            for side in range(2):
                                for c5 in range(3):  # 512-px gather chunks
                                    dst = G[:, side, 4 * c5:4 * c5 + 4, :]
                                    M0 = (th * 3 + c5) * 32
                                    if NOGATHER:
                                        nc.vector.memset(dst, 0.5)
                                    else:
                                        nc.gpsimd.dma_gather(
                                            dst, gather_src,
                                            idxs[side][:, t, M0:M0 + 32],
                                            512, 512, 2 * CIN, elem_step=CIN,
                                        )
                            m = gp.tile([P, 12, CIN], F32, tag="m", name="m")
                            m4 = gp.tile([P, 2, 12, CIN], F32, tag="m4", name="m4")
                            s0 = th * 12
                            for side in range(2):
                                for xc in range(2):
                                    a = cwb[:, side, xc, t, s0:s0 + 12]
                                    # [P, 12 slots, CIN(bcast)] - 3-dim AP
                                    wap = bass.AP(a.tensor, a.offset,
                                                  [a.ap[0], a.ap[1], [0, CIN]])
                                    gs = G[:, side, :, xc * CIN:(xc + 1) * CIN]
                                    nc.vector.tensor_tensor(m4[:, xc], gs, wap,
                                                            op=OP.mult)
                                nc.vector.tensor_tensor(m4[:, 0], m4[:, 0],
                                                        m4[:, 1], op=OP.add)
                                if side == 0:
                                    nc.vector.tensor_copy(m[:], m4[:, 0])
                                else:
                                    nc.vector.tensor_tensor(m[:], m[:], m4[:, 0],
                                                            op=OP.add)
                            if DEBUG and t == 0 and th == 0:
                                for k in range(2):
                                    nc.sync.dma_start(dbg_g[k][:, :12], G[:, k])
                                nc.sync.dma_start(dbg_m[:, :12], m[:])
                            # bounce out in [pix, c]; md[t] is [NPIX, CIN] row-major
                            off = (t * NPIX + th * 12 * P) * CIN
                            nc.sync.dma_start(
                                bass.AP(md.tensor, md.offset + off,
                                        [[CIN, P], [P * CIN, 12], [1, CIN]]),
                                m[:])

                # ---------------- conv matmuls (read back transposed) ----------------
                with (
                    tc.tile_pool(name="mt", bufs=1) as vp,
                    tc.tile_pool(name="acc", bufs=1, space="PSUM") as accp,
                ):
                    for pc in range(3):  # 1536-px output chunks
                        ps = [accp.tile([P, 1536], F32, tag=f"acc{cc}", name=f"acc{cc}")
                              for cc in range(2)]
                        for t in range(NT):
                            mt = vp.tile([P, 2, 1536], F32, tag="mt", name="mt")
                            off = (t * NPIX + pc * 1536) * CIN
                            for kc in range(2):
                                nc.sync.dma_start(
                                    mt[:, kc, :],
                                    bass.AP(md.tensor, md.offset + off + kc * P,
                                            [[1, P], [CIN, 1536]]))
                            if DEBUG and t == 0 and pc == 0:
                                nc.sync.dma_start(dbg_mt[:], mt[:])
                            for cc in range(2):
                                for kc in range(2):
                                    for sub in range(3):
                                        nc.tensor.matmul(
                                            ps[cc][:, sub * 512:(sub + 1) * 512],
                                            wtile[:, t, kc, cc * P:(cc + 1) * P],
                                            mt[:, kc, sub * 512:(sub + 1) * 512],
                                            start=(t == 0 and kc == 0),
                                            stop=(t == NT - 1 and kc == 1),
                                        )
                        for cc in range(2):
                            nc.scalar.copy(out=ysb[:, cc, pc * 1536:(pc + 1) * 1536],
                                           in_=ps[cc][:])

                if DEBUG:
                    nc.sync.dma_start(dbg_y[:], ysb[:])

                # ---------------- GroupNorm ----------------
                with tc.tile_pool(name="gnp", bufs=1, space="PSUM") as gnp, \
                     tc.tile_pool(name="gns", bufs=1) as wk:
                    st = wk.tile([P, 4], F32, tag="st")
                    sq = wk.tile([P, NPIX], F32, tag="sq")
                    for cc in range(2):
                        nc.vector.reduce_sum(st[:, 2 * cc:2 * cc + 1], ysb[:, cc, :], axis=AX.X)
                        nc.vector.tensor_tensor(sq[:], ysb[:, cc, :], ysb[:, cc, :], op=OP.mult)
                        nc.vector.reduce_sum(st[:, 2 * cc + 1:2 * cc + 2], sq[:], axis=AX.X)
                    pg = gnp.tile([8, 4], F32, tag="pg")
                    nc.tensor.matmul(pg[:], ind8[:], st[:], start=True, stop=True)
                    gsb = wk.tile([8, 4], F32, tag="gsb")
                    nc.vector.tensor_copy(gsb[:], pg[:])

                    cind = dp.tile([8, 4], F32, tag="cind")
                    cout_ = dp.tile([8, 4], F32, tag="cout")
                    nc.gpsimd.dma_start(cind[:], gsb[:])
                    if NOCC:
                        nc.sync.dma_start(cout_[:], cind[:])
                    else:
                        nc.gpsimd.collective_compute(
                            "AllReduce", OP.add,
                            replica_groups=[[0, 1], [2, 3], [4, 5], [6, 7]],
                            ins=[cind.opt()], outs=[cout_.opt()],
                        )
                    nc.sync.dma_start(gsb[:], cout_[:])

                    mu = wk.tile([8, 2], F32, tag="mu")
                    e2 = wk.tile([8, 2], F32, tag="e2")
                    nc.vector.tensor_scalar(mu[:], gsb[:, 0::2], 1.0 / NG, None, op0=OP.mult)
                    nc.vector.tensor_scalar(e2[:], gsb[:, 1::2], 1.0 / NG, None, op0=OP.mult)
                    m2t = wk.tile([8, 2], F32, tag="m2t")
                    nc.vector.tensor_tensor(m2t[:], mu[:], mu[:], op=OP.mult)
                    nc.vector.tensor_tensor(e2[:], e2[:], m2t[:], op=OP.subtract)
                    nc.vector.tensor_scalar(e2[:], e2[:], EPS, None, op0=OP.add)
                    rs = wk.tile([8, 2], F32, tag="rs")
                    nc.scalar.activation(rs[:], e2[:], mybir.ActivationFunctionType.Sqrt)
                    nc.vector.reciprocal(rs[:], rs[:])

                    pex = gnp.tile([P, 2], F32, tag="pex")
                    rsc = wk.tile([P, 2], F32, tag="rsc")
                    nc.tensor.matmul(pex[:], e8[:], rs[:], start=True, stop=True)
                    nc.vector.tensor_copy(rsc[:], pex[:])
                    pex2 = gnp.tile([P, 2], F32, tag="pex2")
                    muc = wk.tile([P, 2], F32, tag="muc")
                    nc.tensor.matmul(pex2[:], e8[:], mu[:], start=True, stop=True)
                    nc.vector.tensor_copy(muc[:], pex2[:])

                    sc = wk.tile([P, 2], F32, tag="sc")
                    nc.vector.tensor_tensor(sc[:], rsc[:], gam[:], op=OP.mult)
                    sh = wk.tile([P, 2], F32, tag="sh")
                    nc.vector.tensor_tensor(sh[:], muc[:], sc[:], op=OP.mult)
                    nc.vector.tensor_tensor(sh[:], bet[:], sh[:], op=OP.subtract)

                    for cc in range(2):
                        nc.vector.tensor_scalar(
                            ysb[:, cc, :], ysb[:, cc, :],
                            sc[:, cc:cc + 1], sh[:, cc:cc + 1],
                            op0=OP.mult, op1=OP.add)

            nc.sync.dma_start(yout.rearrange("(cc p) i -> p cc i", p=P), ysb[:])

    nc.compile()
    return nc


def _host_pack(x, offset, mask, weight, gamma, beta):
    """Build the 8 per-core input maps (pure layout work)."""
    in_maps = []
    wts = np.ascontiguousarray(
        weight.reshape(COUT, CIN, 9).transpose(2, 1, 0)).astype(np.float32)
    pgrid = np.arange(P)
    ind8 = (pgrid[:, None] // 16 == np.arange(8)[None, :]).astype(np.float32)
    e8 = np.ascontiguousarray(ind8.T)
    gam2 = np.ascontiguousarray(gamma.reshape(2, P).T).astype(np.float32)
    bet2 = np.ascontiguousarray(beta.reshape(2, P).T).astype(np.float32)

    # pixel index arrays for the two compute layouts
    i_w = np.arange(NPIX).reshape(NS, P).T          # [128, 36]: pix = s*128 + p
    qq, MM = np.meshgrid(np.arange(16), np.arange(MQ), indexing="ij")
    i_q = (MM // 32) * 512 + (MM % 32) * 16 + qq    # [16, 288] wrap-layout pixel
    dy = (np.arange(NT) // 3 - 1).astype(np.float32)
    dx = (np.arange(NT) % 3 - 1).astype(np.float32)

    def grids(ii, h):
        yy = (h * HP + ii // W).astype(np.float32)
        xx = (ii % W).astype(np.float32)
        gy = np.expand_dims(yy, 1) + dy.reshape((1, NT) + (1,) * (ii.ndim - 1))
        gx = np.expand_dims(xx, 1) + dx.reshape((1, NT) + (1,) * (ii.ndim - 1))
        return (np.ascontiguousarray(gy.astype(np.float32)),
                np.ascontiguousarray(gx.astype(np.float32)))

    for core in range(8):
        b, h = core // 2, core % 2
        xtab = np.zeros((R, CIN), dtype=NPDT)
        xtab[1:1 + H * W] = x[b].reshape(CIN, H * W).T.astype(NPDT)
        offs = np.ascontiguousarray(
            offset[b, :, h * HP:(h + 1) * HP, :].reshape(18, NPIX)).astype(np.float32)
        mk = np.ascontiguousarray(
            mask[b, :, h * HP:(h + 1) * HP, :].reshape(NT, NPIX)).astype(np.float32)

        # pixel-partition layout: [P, NT, NS]
        o_pw = offs.reshape(NT, 2, NS, P)
        offs_py = np.ascontiguousarray(o_pw[:, 0].transpose(2, 0, 1))
        offs_px = np.ascontiguousarray(o_pw[:, 1].transpose(2, 0, 1))
        msk_pw = np.ascontiguousarray(mk.reshape(NT, NS, P).transpose(2, 0, 1))
        bw_yv, bw_xv = grids(i_w, h)                # [128, 9, 36]

        # idx-wrap layout: [16, NT, MQ]
        offs_iy = np.ascontiguousarray(offs.reshape(NT, 2, NPIX)[:, 0][:, i_q]
                                       .transpose(1, 0, 2))
        offs_ix = np.ascontiguousarray(offs.reshape(NT, 2, NPIX)[:, 1][:, i_q]
                                       .transpose(1, 0, 2))
        bi_yv, bi_xv = grids(i_q, h)                # [16, 9, 288]
        bi_xv = bi_xv + 1.0  # front-pad row: gather indices are shifted by +1

        in_maps.append({
            "xt": xtab.reshape(-1),
            "offs_py": offs_py, "offs_px": offs_px,
            "msk_pw": msk_pw,
            "bw_y": bw_yv, "bw_x": bw_xv,
            "offs_iy": offs_iy, "offs_ix": offs_ix,
            "bi_y": bi_yv, "bi_x": bi_xv,
            "wtd": wts,
            "ind8d": ind8,
            "e8d": e8,
            "gamd": gam2,
            "betd": bet2,
        })
    return in_maps


def kernel(x, offset, mask, weight, gamma, beta):
    x = np.asarray(x, dtype=np.float32)
    offset = np.asarray(offset, dtype=np.float32)
    mask = np.asarray(mask, dtype=np.float32)
    weight = np.asarray(weight, dtype=np.float32)
    gamma = np.asarray(gamma, dtype=np.float32)
    beta = np.asarray(beta, dtype=np.float32)

    if "nc" not in _CACHED:
        _CACHED["nc"] = _build_nc()
    nc = _CACHED["nc"]

    in_maps = _host_pack(x, offset, mask, weight, gamma, beta)
    res = run_bass_kernel_spmd(nc, in_maps, core_ids=list(range(8)))
    _CACHED["last_results"] = res

    out = np.empty((B, COUT, H, W), dtype=np.float32)
    for core in range(8):
        b, h = core // 2, core % 2
        out[b, :, h * HP:(h + 1) * HP, :] = res.results[core]["y"].reshape(COUT, HP, W)
    return out
